# revision 26
# baseline (speedup 1.0000x reference)
"""AttentionAggregator kernel for 8 TRN2 NeuronCores — j-layout (v3).

Reference computation (per node i over M neighbors j):
    s_self  = self_feats @ a[:D]            # [N]
    s_neigh = features_neighs @ a[D:]       # [M]
    scores  = leaky_relu(s_self[:,None] + s_neigh[None,:], 0.2)
    attn    = softmax(where(mask, scores, -1e30), axis=1); attn = where(mask, attn, 0)
    out     = attn @ features_neighs        # [N, D]

Identity used on device (u-drop): with x = s_i + t_j,
    exp(leaky_relu(x, 0.2)) = u_i * max(v_j, w_i*vh_j),
    u = exp(s_self), w = exp(-0.8 s_self), v = exp(s_neigh), vh = exp(0.2 s_neigh).
u_i > 0 cancels between softmax numerator and denominator, so
    out[i] = (p^T)_i @ F / Z_i,   p[j,i] = m[j,i] * max(w_i*vh_j, v_j),
    Z_i = sum_j p[j,i].

The kernel works entirely TRANSPOSED (j on partitions, i on the free dim),
which removes the per-tile xbar transposes of p (67 MB/core of DMA in the
old i-layout) and turns both per-j factors v_j, vh_j into per-partition
scalars. The mask is applied ADDITIVELY through the mask DMA itself: the
host re-encodes the 0/1 int32 mask as transposed int8 {1: 0, 0: -128}
(a pure re-encoding), the exp arguments are biased by -ln(16) so every
unmasked score factor c is < 128 (softmax cancels the row scale), and the
SWDGE cast DMA lands the mask with accum_op=add — the only compute op the
neuronxcc DMA verifier accepts. Unmasked lanes add +0.0 (exact); masked
lanes go negative and a relu zeroes them.

Per half h (8 node tiles) and j-chunk c (32 chunks of 128 neighbors):
    DVE : ct = (W_b * vh_j) max v_j     (ONE dual-scalar tensor_scalar, 4x)
    DMA : ct += bf16(maskT_i8[c])       (SWDGE cast DMA, accum_op=add)
    DVE : ct = max(ct, 0)               (tensor_scalar, 4x)
    PE  : acc_t[128,129] += ct[:,t,:]^T @ [F_c | 1]  (8 psum banks; the
                                         ones column accumulates Z)
Drain per i-tile: rec = 1/acc[:,128] (DVE), out = acc[:,0:128]*rec (ACT), DMA.

Measured per-pass (slope) HW time ~29-31 us/core vs 87.7 us baseline;
the PE bf16 FLOP floor for this shape is 27.5 us/core.

Sharding: rows of self_feats / columns of maskT split across 8 cores;
features_neighs and `a` replicated. No collectives.
"""

import numpy as np
from contextlib import ExitStack

N, M, D = 16384, 4096, 128
NCORES = 8
NLOC = N // NCORES          # 2048 nodes per core
P = 128                     # partitions
NT = NLOC // P              # 16 node tiles per core
NC_J = M // P               # 32 neighbor chunks

_BUILT = {}


def _build_nc_v3(n_loc=NLOC, m=M, d=D, reps=1, ew="ts2_dma", cbufs=6,
                 nhalves=2, skip_main=False, ablate=(), psum_pack=1):
    """j-layout kernel; mask arrives transposed as int8/uint8 [m, n_loc].

    ew: "add_relu" — mask as int8 {0:-128, 1:0}; SWDGE cast DMA with
                     accum_op=add, then p = relu(c + madd) on DVE. Scores
                     are pre-scaled by 1/256 (softmax row-scale invariant)
                     so unmasked c < 128 and masked c+(-128) < 0. Adding
                     0.0 is exact; the only cce op neuronxcc accepts is add.
        "ts2_dma"  — mask multiply fused into the cast DMA (accum mult;
                     REJECTED by neuronxcc BIR verifier — sim only)
        "ts2_tt"   — plain SWDGE cast DMA (uint8 0/1) + DVE tensor_tensor
                     multiply
    nhalves: split the i range into this many pieces (psum pressure knob).
    """
    import concourse.bass as bass
    import concourse.bacc as bacc
    import concourse.tile as tile
    from concourse import mybir, masks

    f32 = mybir.dt.float32
    bf16 = mybir.dt.bfloat16
    u8 = mybir.dt.uint8
    i8 = mybir.dt.int8
    Op = mybir.AluOpType
    AF = mybir.ActivationFunctionType
    # add_relu: bias the exp args by -ln(16) so unmasked c = e^bias *
    # max(w*vh, v) stays well under 128 (u-dropped c is <~60 unbiased) and
    # masked c + (-128) is always negative; softmax cancels the row scale
    SBIAS = -2.772588722239781 if ew == "add_relu" else 0.0

    nt = n_loc // P
    nj = m // P
    assert nt % nhalves == 0
    tph = nt // nhalves          # i-tiles per psum pass

    nc = bacc.Bacc("TRN2", target_bir_lowering=False, debug=False,
                   num_devices=NCORES)

    self_d = nc.dram_tensor("self_feats", [n_loc, d], f32, kind="ExternalInput")
    f_d = nc.dram_tensor("features_neighs", [m, d], f32, kind="ExternalInput")
    mT_d = nc.dram_tensor("neigh_matrix", [m, n_loc],
                          i8 if ew == "add_relu" else u8,
                          kind="ExternalInput")
    a_d = nc.dram_tensor("a", [2 * d, 1], f32, kind="ExternalInput")
    out_d = nc.dram_tensor("out", [n_loc, d], f32, kind="ExternalOutput")

    with tile.TileContext(nc) as tc, ExitStack() as ctx:
        const = ctx.enter_context(tc.tile_pool(name="const", bufs=1))
        pre_ctx = ExitStack()
        pre = pre_ctx.enter_context(tc.tile_pool(name="pre", bufs=4))
        stage = pre_ctx.enter_context(tc.tile_pool(name="stage", bufs=1))
        psum_pre = pre_ctx.enter_context(
            tc.tile_pool(name="psum_pre", bufs=3, space="PSUM"))
        psum_row = pre_ctx.enter_context(
            tc.tile_pool(name="psum_row", bufs=2, space="PSUM"))

        # self + a load first: they gate the w/W_b chain; F quarters follow
        # on the other HWDGE queue so quarter 0 lands早 and unlocks chunk 0
        self_sb = stage.tile([P, nt, d], f32, tag="selfstage")
        self_src = self_d.ap().rearrange("(t q) d -> q t d", q=P)
        hq = nt // 2
        nc.scalar.dma_start(self_sb[:, 0:hq, :], self_src[:, 0:hq, :])
        nc.scalar.dma_start(self_sb[:, hq:nt, :], self_src[:, hq:nt, :])

        a_self_row = const.tile([1, d], f32)
        a_neigh_row = const.tile([1, d], f32)
        a_flat = a_d.ap().rearrange("(one dd) o2 -> one (dd o2)", one=1)
        nc.scalar.dma_start(a_self_row[:], a_flat[:, 0:d])
        nc.scalar.dma_start(a_neigh_row[:], a_flat[:, d:2 * d])

        f_sb = const.tile([P, nj, d + 1], bf16)
        f_stage = stage.tile([P, nj, d], f32, tag="fstage")
        NQ = 4
        qs = nj // NQ
        f_src = f_d.ap().rearrange("(c q) d -> q c d", q=P)
        for q in range(NQ):
            nc.sync.dma_start(f_stage[:, q * qs:(q + 1) * qs, :],
                              f_src[:, q * qs:(q + 1) * qs, :])

        # ---------------- precompute ----------------
        ident = const.tile([P, P], f32)
        masks.make_identity(nc, ident[:])

        ones1 = const.tile([1, P], f32)
        nc.vector.memset(ones1[:], 1.0)

        # sel4[:, k, :] as weights broadcasts row k of a [4, 128] operand
        sel4 = const.tile([4, 4, P], f32)
        nc.vector.tensor_copy(
            sel4[:], ident[0:4, 0:4].unsqueeze(2).to_broadcast([4, 4, P]))

        a_self_b = const.tile([P, d], f32)
        a_neigh_b = const.tile([P, d], f32)
        for dst, row in ((a_self_b, a_self_row), (a_neigh_b, a_neigh_row)):
            ps = psum_pre.tile([P, d], f32, tag="ps_bc")
            nc.tensor.matmul(ps[:], ones1[:], row[:])
            nc.vector.tensor_copy(dst[:], ps[:])

        a_neigh_bb = const.tile([P, d], bf16)
        nc.vector.tensor_copy(a_neigh_bb[:], a_neigh_b[:])

        # self side: s_self_c[q, t] = s_self[t*128+q]; w = exp(-0.8 s_self).
        # Processed in halves so W_b for the first psum pass is ready before
        # the second half of self_feats even lands.
        s_self_c = const.tile([P, nt], f32)
        w_c = const.tile([P, nt], f32)
        W_b = const.tile([P, nt, P], bf16)
        prod_s = stage.tile([P, nt, d], f32, tag="prod_s")
        for t0 in range(0, nt, hq):
            t1 = t0 + hq
            nc.vector.tensor_tensor(
                prod_s[:, t0:t1, :], self_sb[:, t0:t1, :],
                a_self_b[:].unsqueeze(1).to_broadcast([P, hq, d]), Op.mult)
            nc.vector.tensor_reduce(s_self_c[:, t0:t1],
                                    prod_s[:, t0:t1, :],
                                    mybir.AxisListType.X, Op.add)
            nc.scalar.activation(w_c[:, t0:t1], s_self_c[:, t0:t1],
                                 AF.Exp, scale=-0.8)
            # broadcast w across partitions: W_b[j, t*128+q] = w[q, t]
            for q0 in range(t0, t1, 4):
                pst4 = psum_row.tile([4, P], f32, tag="ps_row")
                nc.tensor.transpose(pst4[:], w_c[:, q0:q0 + 4], ident[:])
                rows4 = pre.tile([4, P], f32, tag="pre_row")
                nc.vector.tensor_copy(rows4[:], pst4[:])
                bank = psum_pre.tile([P, 4 * P], f32, tag="ps_bank")
                for k in range(4):
                    nc.tensor.matmul(bank[:, k * P:(k + 1) * P],
                                     sel4[:, k, :], rows4[:])
                nc.scalar.copy(
                    W_b[:, q0:q0 + 4, :].rearrange("p c q -> p (c q)"),
                    bank[:])

        # neighbor side: s_neigh_c[j', c] = t_{c*128+j'}; v/vh per-partition.
        # Per-quarter exps so the scheduler can start main-loop chunks of
        # quarter 0 while quarters 1-3 are still loading/dotting; the
        # f32->bf16 F cast runs on ACT to keep the DVE queue short.
        s_neigh_c = const.tile([P, nj], f32)
        v_c = const.tile([P, nj], f32)
        vh_c = const.tile([P, nj], f32)
        sbias_t = const.tile([P, 1], f32)
        nc.vector.memset(sbias_t[:], SBIAS)
        prod_n = stage.tile([P, nj, d], bf16, tag="prod_n")
        for q in range(NQ):
            lo_q, hi_q = q * qs, (q + 1) * qs
            nc.scalar.copy(f_sb[:, lo_q:hi_q, 0:d],
                           f_stage[:, lo_q:hi_q, :])
            nc.vector.tensor_tensor(
                prod_n[:, lo_q:hi_q, :], f_sb[:, lo_q:hi_q, 0:d],
                a_neigh_bb[:].unsqueeze(1).to_broadcast([P, qs, d]), Op.mult)
            nc.vector.tensor_reduce(s_neigh_c[:, lo_q:hi_q],
                                    prod_n[:, lo_q:hi_q, :],
                                    mybir.AxisListType.X, Op.add)
            nc.scalar.activation(v_c[:, lo_q:hi_q], s_neigh_c[:, lo_q:hi_q],
                                 AF.Exp, bias=sbias_t[:])
            nc.scalar.activation(vh_c[:, lo_q:hi_q], s_neigh_c[:, lo_q:hi_q],
                                 AF.Exp, scale=0.2, bias=sbias_t[:])

        # ones column per chunk: f_sb flat index c*(d+1)+d
        nc.gpsimd.memset(f_sb[:].rearrange("p c q -> p (c q)")
                         [:, d::d + 1], 1.0)

        pre_ctx.close()  # release precompute SBUF/PSUM pools

        cpool = ctx.enter_context(tc.tile_pool(name="cpool", bufs=cbufs))
        if ew == "ts2_tt":
            mpool = ctx.enter_context(tc.tile_pool(name="mpool", bufs=cbufs))
        # psum_pack=2: two [P,129] accumulators share one 2KB bank, so two
        # half-passes' accumulators coexist and PE never stalls on drains
        psum_mm = ctx.enter_context(
            tc.tile_pool(name="psum_mm",
                         bufs=(2 * tph + psum_pack - 1) // psum_pack
                         if psum_pack > 1 else tph,
                         space="PSUM"))
        outp = ctx.enter_context(tc.tile_pool(name="outp", bufs=4))
        small = ctx.enter_context(tc.tile_pool(name="small", bufs=8))

        if skip_main:
            o_t = outp.tile([P, d], f32)
            nc.vector.memset(o_t[:], 0.0)
            nc.sync.dma_start(out_d[0:P, :], o_t[:])
        else:
            for rep in range(reps):
                for h in range(nhalves):
                    i_lo = h * tph * P          # node range of this pass
                    accs = []
                    if "mm" not in ablate and psum_pack > 1:
                        for t0 in range(0, tph, psum_pack):
                            acc_pk = psum_mm.tile([P, psum_pack, d + 1],
                                                  f32, tag="acc")
                            for k in range(min(psum_pack, tph - t0)):
                                accs.append(acc_pk[:, k, :])
                    elif "mm" not in ablate:
                        for t in range(tph):
                            acc_t = psum_mm.tile([P, d + 1], f32, tag="acc")
                            accs.append(acc_t)
                    for c in range(nj):
                        ct = cpool.tile([P, tph, P], bf16, tag="c")
                        ct_flat = ct[:].rearrange("p t q -> p (t q)")
                        wb_flat = (W_b[:, h * tph:(h + 1) * tph, :]
                                   .rearrange("p t q -> p (t q)"))
                        if "ts2" not in ablate:
                            nc.vector.tensor_scalar(
                                ct_flat, wb_flat, vh_c[:, c:c + 1],
                                v_c[:, c:c + 1], Op.mult, Op.max)
                        msrc = mT_d[c * P:(c + 1) * P,
                                    i_lo:i_lo + tph * P]
                        if "mask" in ablate:
                            pass
                        elif ew == "add_relu":
                            # masked lanes get += -128 (c < 128 so they go
                            # negative); unmasked lanes get += 0.0 exactly
                            nc.gpsimd.dma_start(ct_flat, msrc,
                                                accum_op=Op.add)
                            if "relu" not in ablate:
                                nc.vector.tensor_scalar(
                                    ct_flat, ct_flat, 0.0, None, Op.max)
                        elif ew == "ts2_dma":
                            # mask multiply fused into the cast DMA
                            nc.gpsimd.dma_start(ct_flat, msrc,
                                                accum_op=Op.mult)
                        else:
                            mt = mpool.tile([P, tph * P], bf16, tag="m")
                            nc.gpsimd.dma_start(mt[:], msrc)
                            nc.vector.tensor_tensor(ct_flat, ct_flat, mt[:],
                                                    Op.mult)
                        if "mm" not in ablate:
                            for t in range(tph):
                                nc.tensor.matmul(accs[t][:], ct[:, t, :],
                                                 f_sb[:, c, :],
                                                 start=(c == 0),
                                                 stop=(c == nj - 1))
                    for t in range(tph):
                        gt = h * tph + t
                        o_t = outp.tile([P, d], f32, tag="of")
                        if "mm" in ablate:
                            nc.vector.memset(o_t[:], 0.0)
                        else:
                            rec = small.tile([P, 1], f32, tag="rec")
                            nc.vector.reciprocal(rec[:], accs[t][:, d:d + 1])
                            nc.scalar.mul(o_t[:], accs[t][:, 0:d], rec[:])
                        nc.scalar.dma_start(out_d[gt * P:(gt + 1) * P, :],
                                            o_t[:])

    nc.compile()
    return nc


EW_DEFAULT = "add_relu"


def _get_nc(key=None):
    if key is None:
        key = (EW_DEFAULT, 1)
    if key not in _BUILT:
        _BUILT[key] = _build_nc_v3(reps=key[1], ew=key[0])
    return _BUILT[key]


def _encode_mask(neigh_matrix, ew=None):
    """Host-side re-encoding of the 0/1 int32 mask for the device kernel."""
    if ew is None:
        ew = EW_DEFAULT
    if ew == "add_relu":
        # {1: 0, 0: -128} int8 additive mask
        return ((neigh_matrix.astype(np.int32) - 1) * 128).astype(np.int8)
    return neigh_matrix.astype(np.uint8)


def kernel(self_feats, features_neighs, neigh_matrix, a):
    from concourse.bass_utils import run_bass_kernel_spmd

    self_feats = np.ascontiguousarray(self_feats, dtype=np.float32)
    features_neighs = np.ascontiguousarray(features_neighs, dtype=np.float32)
    a = np.ascontiguousarray(a, dtype=np.float32)
    # host-side re-encoding: transpose + narrow the 0/1 mask to 1 byte
    m8 = _encode_mask(neigh_matrix)

    nc = _get_nc()
    in_maps = []
    for c in range(NCORES):
        sl = slice(c * NLOC, (c + 1) * NLOC)
        in_maps.append({
            "self_feats": self_feats[sl],
            "features_neighs": features_neighs,
            "neigh_matrix": np.ascontiguousarray(m8[sl].T),
            "a": a,
        })
    res = run_bass_kernel_spmd(nc, in_maps, core_ids=list(range(NCORES)))
    out = np.concatenate([np.asarray(res.results[c]["out"])
                          for c in range(NCORES)], axis=0)
    return out.astype(np.float32)


# revision 27
# speedup vs baseline: 4.2325x; 4.2325x over previous
"""AttentionAggregator kernel for 8 TRN2 NeuronCores — j-layout (v3).

Reference computation (per node i over M neighbors j):
    s_self  = self_feats @ a[:D]            # [N]
    s_neigh = features_neighs @ a[D:]       # [M]
    scores  = leaky_relu(s_self[:,None] + s_neigh[None,:], 0.2)
    attn    = softmax(where(mask, scores, -1e30), axis=1); attn = where(mask, attn, 0)
    out     = attn @ features_neighs        # [N, D]

Identity used on device (u-drop): with x = s_i + t_j,
    exp(leaky_relu(x, 0.2)) = u_i * max(v_j, w_i*vh_j),
    u = exp(s_self), w = exp(-0.8 s_self), v = exp(s_neigh), vh = exp(0.2 s_neigh).
u_i > 0 cancels between softmax numerator and denominator, so
    out[i] = (p^T)_i @ F / Z_i,   p[j,i] = m[j,i] * max(w_i*vh_j, v_j),
    Z_i = sum_j p[j,i].

The kernel works entirely TRANSPOSED (j on partitions, i on the free dim),
which removes the per-tile xbar transposes of p (67 MB/core of DMA in the
old i-layout) and turns both per-j factors v_j, vh_j into per-partition
scalars. The mask is applied ADDITIVELY through the mask DMA itself: the
host re-encodes the 0/1 int32 mask as transposed int8 {1: 0, 0: -128}
(a pure re-encoding), the exp arguments are biased by -ln(16) so every
unmasked score factor c is < 128 (softmax cancels the row scale), and the
SWDGE cast DMA lands the mask with accum_op=add — the only compute op the
neuronxcc DMA verifier accepts. Unmasked lanes add +0.0 (exact); masked
lanes go negative and a relu zeroes them.

Per half h (8 node tiles) and j-chunk c (32 chunks of 128 neighbors):
    DVE : ct = (W_b * vh_j) max v_j     (ONE dual-scalar tensor_scalar, 4x)
    DMA : ct += bf16(maskT_i8[c])       (SWDGE cast DMA, accum_op=add)
    DVE : ct = max(ct, 0)               (tensor_scalar, 4x)
    PE  : acc_t[128,129] += ct[:,t,:]^T @ [F_c | 1]  (8 psum banks; the
                                         ones column accumulates Z)
Drain per i-tile: rec = 1/acc[:,128] (DVE), out = acc[:,0:128]*rec (ACT), DMA.

Measured per-pass (slope) HW time ~29-31 us/core vs 87.7 us baseline;
the PE bf16 FLOP floor for this shape is 27.5 us/core.

Sharding: rows of self_feats / columns of maskT split across 8 cores;
features_neighs and `a` replicated. No collectives.
"""

import numpy as np
from contextlib import ExitStack

N, M, D = 16384, 4096, 128
NCORES = 8
NLOC = N // NCORES          # 2048 nodes per core
P = 128                     # partitions
NT = NLOC // P              # 16 node tiles per core
NC_J = M // P               # 32 neighbor chunks

_BUILT = {}


def _build_nc_v3(n_loc=NLOC, m=M, d=D, reps=1, ew="ts2_dma", cbufs=6,
                 nhalves=2, skip_main=False, ablate=(), psum_pack=1):
    """j-layout kernel; mask arrives transposed as int8/uint8 [m, n_loc].

    ew: "add_relu" — mask as int8 {0:-128, 1:0}; SWDGE cast DMA with
                     accum_op=add, then p = relu(c + madd) on DVE. Scores
                     are pre-scaled by 1/256 (softmax row-scale invariant)
                     so unmasked c < 128 and masked c+(-128) < 0. Adding
                     0.0 is exact; the only cce op neuronxcc accepts is add.
        "ts2_dma"  — mask multiply fused into the cast DMA (accum mult;
                     REJECTED by neuronxcc BIR verifier — sim only)
        "ts2_tt"   — plain SWDGE cast DMA (uint8 0/1) + DVE tensor_tensor
                     multiply
    nhalves: split the i range into this many pieces (psum pressure knob).
    """
    import concourse.bass as bass
    import concourse.bacc as bacc
    import concourse.tile as tile
    from concourse import mybir, masks

    f32 = mybir.dt.float32
    bf16 = mybir.dt.bfloat16
    u8 = mybir.dt.uint8
    i8 = mybir.dt.int8
    Op = mybir.AluOpType
    AF = mybir.ActivationFunctionType
    # add_relu: bias the exp args by -ln(16) so unmasked c = e^bias *
    # max(w*vh, v) stays well under 128 (u-dropped c is <~60 unbiased) and
    # masked c + (-128) is always negative; softmax cancels the row scale
    SBIAS = -2.772588722239781 if ew == "add_relu" else 0.0

    nt = n_loc // P
    nj = m // P
    assert nt % nhalves == 0
    tph = nt // nhalves          # i-tiles per psum pass

    nc = bacc.Bacc("TRN2", target_bir_lowering=False, debug=False,
                   num_devices=NCORES)

    self_d = nc.dram_tensor("self_feats", [n_loc, d], f32, kind="ExternalInput")
    f_d = nc.dram_tensor("features_neighs", [m, d], f32, kind="ExternalInput")
    mT_d = nc.dram_tensor("neigh_matrix", [m, n_loc],
                          i8 if ew == "add_relu" else u8,
                          kind="ExternalInput")
    a_d = nc.dram_tensor("a", [2 * d, 1], f32, kind="ExternalInput")
    out_d = nc.dram_tensor("out", [n_loc, d], f32, kind="ExternalOutput")

    with tile.TileContext(nc) as tc, ExitStack() as ctx:
        const = ctx.enter_context(tc.tile_pool(name="const", bufs=1))
        pre_ctx = ExitStack()
        pre = pre_ctx.enter_context(tc.tile_pool(name="pre", bufs=4))
        stage = pre_ctx.enter_context(tc.tile_pool(name="stage", bufs=1))
        psum_pre = pre_ctx.enter_context(
            tc.tile_pool(name="psum_pre", bufs=3, space="PSUM"))
        psum_row = pre_ctx.enter_context(
            tc.tile_pool(name="psum_row", bufs=2, space="PSUM"))

        # self + a load first: they gate the w/W_b chain; F quarters follow
        # on the other HWDGE queue so quarter 0 lands早 and unlocks chunk 0
        self_sb = stage.tile([P, nt, d], f32, tag="selfstage")
        self_src = self_d.ap().rearrange("(t q) d -> q t d", q=P)
        hq = nt // 2
        nc.scalar.dma_start(self_sb[:, 0:hq, :], self_src[:, 0:hq, :])
        nc.scalar.dma_start(self_sb[:, hq:nt, :], self_src[:, hq:nt, :])

        a_self_row = const.tile([1, d], f32)
        a_neigh_row = const.tile([1, d], f32)
        a_flat = a_d.ap().rearrange("(one dd) o2 -> one (dd o2)", one=1)
        nc.scalar.dma_start(a_self_row[:], a_flat[:, 0:d])
        nc.scalar.dma_start(a_neigh_row[:], a_flat[:, d:2 * d])

        f_sb = const.tile([P, nj, d + 1], bf16)
        f_stage = stage.tile([P, nj, d], f32, tag="fstage")
        NQ = 4
        qs = nj // NQ
        f_src = f_d.ap().rearrange("(c q) d -> q c d", q=P)
        for q in range(NQ):
            nc.sync.dma_start(f_stage[:, q * qs:(q + 1) * qs, :],
                              f_src[:, q * qs:(q + 1) * qs, :])

        # ---------------- precompute ----------------
        ident = const.tile([P, P], f32)
        masks.make_identity(nc, ident[:])

        ones1 = const.tile([1, P], f32)
        nc.vector.memset(ones1[:], 1.0)

        # sel4[:, k, :] as weights broadcasts row k of a [4, 128] operand
        sel4 = const.tile([4, 4, P], f32)
        nc.vector.tensor_copy(
            sel4[:], ident[0:4, 0:4].unsqueeze(2).to_broadcast([4, 4, P]))

        a_self_b = const.tile([P, d], f32)
        a_neigh_b = const.tile([P, d], f32)
        for dst, row in ((a_self_b, a_self_row), (a_neigh_b, a_neigh_row)):
            ps = psum_pre.tile([P, d], f32, tag="ps_bc")
            nc.tensor.matmul(ps[:], ones1[:], row[:])
            nc.vector.tensor_copy(dst[:], ps[:])

        a_neigh_bb = const.tile([P, d], bf16)
        nc.vector.tensor_copy(a_neigh_bb[:], a_neigh_b[:])

        # self side: s_self_c[q, t] = s_self[t*128+q]; w = exp(-0.8 s_self).
        # Processed in halves so W_b for the first psum pass is ready before
        # the second half of self_feats even lands.
        s_self_c = const.tile([P, nt], f32)
        w_c = const.tile([P, nt], f32)
        W_b = const.tile([P, nt, P], bf16)
        prod_s = stage.tile([P, nt, d], f32, tag="prod_s")
        for t0 in range(0, nt, hq):
            t1 = t0 + hq
            nc.vector.tensor_tensor(
                prod_s[:, t0:t1, :], self_sb[:, t0:t1, :],
                a_self_b[:].unsqueeze(1).to_broadcast([P, hq, d]), Op.mult)
            nc.vector.tensor_reduce(s_self_c[:, t0:t1],
                                    prod_s[:, t0:t1, :],
                                    mybir.AxisListType.X, Op.add)
            nc.scalar.activation(w_c[:, t0:t1], s_self_c[:, t0:t1],
                                 AF.Exp, scale=-0.8)
            # broadcast w across partitions: W_b[j, t*128+q] = w[q, t]
            for q0 in range(t0, t1, 4):
                pst4 = psum_row.tile([4, P], f32, tag="ps_row")
                nc.tensor.transpose(pst4[:], w_c[:, q0:q0 + 4], ident[:])
                rows4 = pre.tile([4, P], f32, tag="pre_row")
                nc.vector.tensor_copy(rows4[:], pst4[:])
                bank = psum_pre.tile([P, 4 * P], f32, tag="ps_bank")
                for k in range(4):
                    nc.tensor.matmul(bank[:, k * P:(k + 1) * P],
                                     sel4[:, k, :], rows4[:])
                nc.scalar.copy(
                    W_b[:, q0:q0 + 4, :].rearrange("p c q -> p (c q)"),
                    bank[:])

        # neighbor side: s_neigh_c[j', c] = t_{c*128+j'}; v/vh per-partition.
        # Per-quarter exps so the scheduler can start main-loop chunks of
        # quarter 0 while quarters 1-3 are still loading/dotting; the
        # f32->bf16 F cast runs on ACT to keep the DVE queue short.
        s_neigh_c = const.tile([P, nj], f32)
        v_c = const.tile([P, nj], f32)
        vh_c = const.tile([P, nj], f32)
        sbias_t = const.tile([P, 1], f32)
        nc.vector.memset(sbias_t[:], SBIAS)
        prod_n = stage.tile([P, nj, d], bf16, tag="prod_n")
        for q in range(NQ):
            lo_q, hi_q = q * qs, (q + 1) * qs
            nc.scalar.copy(f_sb[:, lo_q:hi_q, 0:d],
                           f_stage[:, lo_q:hi_q, :])
            nc.vector.tensor_tensor(
                prod_n[:, lo_q:hi_q, :], f_sb[:, lo_q:hi_q, 0:d],
                a_neigh_bb[:].unsqueeze(1).to_broadcast([P, qs, d]), Op.mult)
            nc.vector.tensor_reduce(s_neigh_c[:, lo_q:hi_q],
                                    prod_n[:, lo_q:hi_q, :],
                                    mybir.AxisListType.X, Op.add)
            nc.scalar.activation(v_c[:, lo_q:hi_q], s_neigh_c[:, lo_q:hi_q],
                                 AF.Exp, bias=sbias_t[:])
            nc.scalar.activation(vh_c[:, lo_q:hi_q], s_neigh_c[:, lo_q:hi_q],
                                 AF.Exp, scale=0.2, bias=sbias_t[:])

        # ones column per chunk: f_sb flat index c*(d+1)+d
        nc.gpsimd.memset(f_sb[:].rearrange("p c q -> p (c q)")
                         [:, d::d + 1], 1.0)

        pre_ctx.close()  # release precompute SBUF/PSUM pools

        cpool = ctx.enter_context(tc.tile_pool(name="cpool", bufs=cbufs))
        if ew == "ts2_tt":
            mpool = ctx.enter_context(tc.tile_pool(name="mpool", bufs=cbufs))
        # Give the accumulator pool up to 8 banks: with tph < 8 two passes'
        # accumulators coexist (double-buffered) and PE never stalls on
        # drains at pass boundaries. NOTE psum_pack=2 (two [P,129] tiles
        # sharing one bank) is numerically WRONG on HW: the matmul start
        # bit zeroes the whole bank, corrupting the co-resident tile.
        psum_mm = ctx.enter_context(
            tc.tile_pool(name="psum_mm",
                         bufs=(2 * tph + psum_pack - 1) // psum_pack
                         if psum_pack > 1 else min(8, 2 * tph),
                         space="PSUM"))
        outp = ctx.enter_context(tc.tile_pool(name="outp", bufs=4))
        small = ctx.enter_context(tc.tile_pool(name="small", bufs=8))

        if skip_main:
            o_t = outp.tile([P, d], f32)
            nc.vector.memset(o_t[:], 0.0)
            nc.sync.dma_start(out_d[0:P, :], o_t[:])
        else:
            for rep in range(reps):
                for h in range(nhalves):
                    i_lo = h * tph * P          # node range of this pass
                    accs = []
                    if "mm" not in ablate and psum_pack > 1:
                        for t0 in range(0, tph, psum_pack):
                            acc_pk = psum_mm.tile([P, psum_pack, d + 1],
                                                  f32, tag="acc")
                            for k in range(min(psum_pack, tph - t0)):
                                accs.append(acc_pk[:, k, :])
                    elif "mm" not in ablate:
                        for t in range(tph):
                            acc_t = psum_mm.tile([P, d + 1], f32, tag="acc")
                            accs.append(acc_t)
                    for c in range(nj):
                        ct = cpool.tile([P, tph, P], bf16, tag="c")
                        ct_flat = ct[:].rearrange("p t q -> p (t q)")
                        wb_flat = (W_b[:, h * tph:(h + 1) * tph, :]
                                   .rearrange("p t q -> p (t q)"))
                        if "ts2" not in ablate:
                            nc.vector.tensor_scalar(
                                ct_flat, wb_flat, vh_c[:, c:c + 1],
                                v_c[:, c:c + 1], Op.mult, Op.max)
                        msrc = mT_d[c * P:(c + 1) * P,
                                    i_lo:i_lo + tph * P]
                        if "mask" in ablate:
                            pass
                        elif ew == "add_relu":
                            # masked lanes get += -128 (c < 128 so they go
                            # negative); unmasked lanes get += 0.0 exactly
                            nc.gpsimd.dma_start(ct_flat, msrc,
                                                accum_op=Op.add)
                            if "relu" not in ablate:
                                nc.vector.tensor_scalar(
                                    ct_flat, ct_flat, 0.0, None, Op.max)
                        elif ew == "ts2_dma":
                            # mask multiply fused into the cast DMA
                            nc.gpsimd.dma_start(ct_flat, msrc,
                                                accum_op=Op.mult)
                        else:
                            mt = mpool.tile([P, tph * P], bf16, tag="m")
                            nc.gpsimd.dma_start(mt[:], msrc)
                            nc.vector.tensor_tensor(ct_flat, ct_flat, mt[:],
                                                    Op.mult)
                        if "mm" not in ablate:
                            for t in range(tph):
                                nc.tensor.matmul(accs[t][:], ct[:, t, :],
                                                 f_sb[:, c, :],
                                                 start=(c == 0),
                                                 stop=(c == nj - 1))
                    for t in range(tph):
                        gt = h * tph + t
                        o_t = outp.tile([P, d], f32, tag="of")
                        if "mm" in ablate:
                            nc.vector.memset(o_t[:], 0.0)
                        else:
                            rec = small.tile([P, 1], f32, tag="rec")
                            nc.vector.reciprocal(rec[:], accs[t][:, d:d + 1])
                            nc.scalar.mul(o_t[:], accs[t][:, 0:d], rec[:])
                        nc.scalar.dma_start(out_d[gt * P:(gt + 1) * P, :],
                                            o_t[:])

    nc.compile()
    return nc


EW_DEFAULT = "add_relu"


def _get_nc(key=None):
    if key is None:
        key = (EW_DEFAULT, 1)
    if key not in _BUILT:
        _BUILT[key] = _build_nc_v3(reps=key[1], ew=key[0])
    return _BUILT[key]


def _encode_mask(neigh_matrix, ew=None):
    """Host-side re-encoding of the 0/1 int32 mask for the device kernel."""
    if ew is None:
        ew = EW_DEFAULT
    if ew == "add_relu":
        # {1: 0, 0: -128} int8 additive mask
        return ((neigh_matrix.astype(np.int32) - 1) * 128).astype(np.int8)
    return neigh_matrix.astype(np.uint8)


def kernel(self_feats, features_neighs, neigh_matrix, a):
    from concourse.bass_utils import run_bass_kernel_spmd

    self_feats = np.ascontiguousarray(self_feats, dtype=np.float32)
    features_neighs = np.ascontiguousarray(features_neighs, dtype=np.float32)
    a = np.ascontiguousarray(a, dtype=np.float32)
    # host-side re-encoding: transpose + narrow the 0/1 mask to 1 byte
    m8 = _encode_mask(neigh_matrix)

    nc = _get_nc()
    in_maps = []
    for c in range(NCORES):
        sl = slice(c * NLOC, (c + 1) * NLOC)
        in_maps.append({
            "self_feats": self_feats[sl],
            "features_neighs": features_neighs,
            "neigh_matrix": np.ascontiguousarray(m8[sl].T),
            "a": a,
        })
    res = run_bass_kernel_spmd(nc, in_maps, core_ids=list(range(NCORES)))
    out = np.concatenate([np.asarray(res.results[c]["out"])
                          for c in range(NCORES)], axis=0)
    return out.astype(np.float32)
